# revision 56
# baseline (speedup 1.0000x reference)
"""DotProductNonLocalBlock Trainium2 kernel.

Reference computation (per batch b, with xf = x[b] reshaped (C, N)):
    Q = wq @ xf + bq ; K = wk @ xf + bk ; V = wv @ xf + bv      (E x N each)
    attn  = (Q^T K) / N                                          (N x N)
    embed = attn @ V^T                                           (N x E)
    out   = wo @ embed^T + bo + xf                               (C x N)

There is no softmax, so matmul associativity collapses the N x N
attention matrix:
    KV   = K @ V^T                          (E x E)
    out  = wo @ (KV^T @ Q) / N + bo + xf
         = W1 @ xf + cvec + xf,   W1 = wo @ KV^T @ wq / N  (C x C)
         with cvec = wo @ (KV^T @ bq) / N + bo
This turns ~13.4 GFLOP/batch into ~3.7 GFLOP/batch of dense matmuls.

Matmuls run in float32r (full PE rate; plain fp32 is 4 cycles/row).
float32r operands must be produced by a rounding instruction, so every
matmul input tile is written by DVE/ACT with a float32r output dtype.

Sharding: data-parallel over batch, one batch per NeuronCore (8 cores).
"""

import numpy as np

import concourse.bass as bass
from concourse import bacc
import concourse.mybir as mybir
import concourse.tile as tile
from concourse.bass_utils import run_bass_kernel_spmd
from concourse.masks import make_identity

F32 = mybir.dt.float32
F32R = mybir.dt.float32r
ACT_COPY = mybir.ActivationFunctionType.Copy
ACT_IDENT = mybir.ActivationFunctionType.Identity

P = 128          # partitions
C = 512          # in_channels
E = 256          # embed dim
N = 3136         # 56*56 pixels
NP = 3200        # N padded to 25*128
CB = C // P      # 4 channel blocks
EB = E // P      # 2 embed blocks
NBLK = NP // P   # 25 pixel blocks of 128
NT = 7           # pixel tiles of 448 for projection-style matmuls
NTW = N // NT    # 448


def build_kernel():
    nc = bacc.Bacc(None, target_bir_lowering=False)

    x_d = nc.dram_tensor("x", (C, N), F32, kind="ExternalInput")
    wq_d = nc.dram_tensor("wq", (E, C), F32, kind="ExternalInput")
    bq_d = nc.dram_tensor("bq", (E,), F32, kind="ExternalInput")
    wk_d = nc.dram_tensor("wk", (E, C), F32, kind="ExternalInput")
    bk_d = nc.dram_tensor("bk", (E,), F32, kind="ExternalInput")
    wv_d = nc.dram_tensor("wv", (E, C), F32, kind="ExternalInput")
    bv_d = nc.dram_tensor("bv", (E,), F32, kind="ExternalInput")
    wo_d = nc.dram_tensor("wo", (C, E), F32, kind="ExternalInput")
    bo_d = nc.dram_tensor("bo", (C,), F32, kind="ExternalInput")
    out_d = nc.dram_tensor("out", (C, N), F32, kind="ExternalOutput")

    xr = x_d[:, :].rearrange("(cb p) n -> p cb n", p=P)
    our = out_d[:, :].rearrange("(cb p) n -> p cb n", p=P)

    with tile.TileContext(nc) as tc:
        with (
            tc.tile_pool(name="const", bufs=1) as cpool,
            tc.tile_pool(name="work", bufs=3) as wpool,
            tc.tile_pool(name="psA", bufs=3, space="PSUM") as psA,
            tc.tile_pool(name="psB", bufs=3, space="PSUM") as psB,
        ):
            # ---------------- constants & inputs in SBUF ----------------
            # bkv_row memset must precede ident/e0 so the Pool-observing
            # transpose below covers its tick too (PE matmuls allow 1 wait).
            bkv_row = cpool.tile([P, 2 * E], F32, tag="bkv_row")
            nc.gpsimd.memset(bkv_row, 0.0)
            ident = cpool.tile([P, P], F32, tag="ident")
            make_identity(nc, ident)

            # DMA issue order matters: the shared DMA backend serializes, so
            # load what gates compute first (wk/wv for the weight transposes,
            # then x in small leading chunks), and stage-2-only weights last.
            # x tiles declared up front; loads are interleaved with the
            # weight DMAs below (the shared DMA backend serializes, so the
            # issue order IS the arrival order).
            xf = cpool.tile([P, CB, NP], F32, tag="xf")
            xfr = cpool.tile([P, CB, NP], F32R, tag="xfr")
            nc.gpsimd.memset(xf[:, :, N:NP], 0.0)
            nc.scalar.activation(xfr[:, :, N:NP], xf[:, :, N:NP], ACT_COPY)

            def load_x(a, b):
                nc.sync.dma_start(xf[:, :, a:b], xr[:, :, a:b])
                # rounding copy on ACT: keeps DVE free for stage-1 evacuations
                nc.scalar.activation(xfr[:, :, a:b], xf[:, :, a:b], ACT_COPY)

            wk_nat = cpool.tile([P, EB, C], F32, tag="wk_nat")
            wv_nat = cpool.tile([P, EB, C], F32, tag="wv_nat")
            for eb in range(EB):
                nc.sync.dma_start(
                    wk_nat[:, eb], wk_d[:, :].rearrange("(eb p) c -> p eb c", p=P)[:, eb]
                )
            load_x(0, P)
            load_x(P, 2 * P)
            for eb in range(EB):
                nc.sync.dma_start(
                    wv_nat[:, eb], wv_d[:, :].rearrange("(eb p) c -> p eb c", p=P)[:, eb]
                )
            load_x(2 * P, 3 * P)
            load_x(3 * P, 4 * P)
            nc.sync.dma_start(bkv_row[0:1, 0:E], bk_d[:].rearrange("(o e) -> o e", o=1))
            nc.sync.dma_start(bkv_row[0:1, E:2 * E], bv_d[:].rearrange("(o e) -> o e", o=1))
            for i in range(6):
                a = 512 + i * NTW
                b = min(a + NTW, N)
                load_x(a, b)

            # stage-2-only weights, loaded behind x
            wq_sb = cpool.tile([P, EB, C], F32, tag="wq_sb")
            nc.sync.dma_start(wq_sb, wq_d[:, :].rearrange("(eb p) c -> p eb c", p=P))
            wo_sb = cpool.tile([P, CB, E], F32, tag="wo_sb")
            nc.sync.dma_start(wo_sb, wo_d[:, :].rearrange("(cb p) e -> p cb e", p=P))
            bq_sb = cpool.tile([P, EB], F32, tag="bq_sb")
            nc.sync.dma_start(bq_sb, bq_d[:].rearrange("(eb p) -> p eb", p=P))
            bo_sb = cpool.tile([P, CB], F32, tag="bo_sb")
            nc.sync.dma_start(bo_sb, bo_d[:].rearrange("(cb p) -> p cb", p=P))

            # ------------- weight transposes via PE (fp32) -------------
            # wkvT[c, cb, 0:E]=wk^T, [c, cb, E:2E]=wv^T  (contraction layout)
            wkvT = cpool.tile([P, CB, 2 * E], F32R, tag="wkvT")

            def transpose_into(src, off):
                for eb in range(EB):
                    for cb in range(CB):
                        # borrow the (idle) stage-3 psum slots: 4-deep rotation
                        # keeps the transpose->copy pipeline streaming
                        tp = psB.tile([P, NTW], F32, tag="ps3", name="tp")
                        nc.tensor.transpose(
                            tp[:, :P], src[:, eb, cb * P:(cb + 1) * P], ident
                        )
                        dst = wkvT[:, cb, off + eb * P:off + (eb + 1) * P]
                        if cb % 2 == 0:
                            nc.vector.tensor_copy(dst, tp[:, :P])
                        else:
                            nc.scalar.activation(dst, tp[:, :P], ACT_COPY)

            transpose_into(wk_nat, 0)
            transpose_into(wv_nat, E)

            # [bk|bv] replicated on all partitions via GpSimd broadcast
            # (keeps PE free and off the bias-DMA dependency)
            bkv_full = cpool.tile([P, 2 * E], F32, tag="bkv_full")
            nc.gpsimd.partition_broadcast(bkv_full, bkv_row[0:1, :])

            # ------------- stage 1: K^T/V^T blocks + KV accumulation -------------
            # For each 128-pixel block: psum[n, 0:E]=K^T, [n, E:2E]=V^T
            # then KV[e, eh] += KT_blk^T @ VT_blk  accumulated in PSUM.
            kv_ps = []
            for eb in range(EB):
                kvt = psA.tile([P, E], F32, tag="kv_ps", bufs=EB, name="kvt")
                kv_ps.append(kvt)
            woT = cpool.tile([P, EB, C], F32R, tag="woT")
            wq_r = cpool.tile([P, EB, C], F32R, tag="wq_r")
            bq_r = cpool.tile([P, EB, 2], F32R, tag="bq_r")
            # the last pixel block has only 64 valid rows; its padded rows
            # must be exactly zero for KV. Pre-zero a dedicated tile early so
            # only the valid-row add sits on the end-of-stage-1 critical path.
            ktvt_last = cpool.tile([P, 2 * E], F32R, tag="ktvt_last")
            nc.vector.tensor_scalar_mul(ktvt_last[64:], bkv_full[64:], 0.0)
            for nb in range(NBLK):
                if 14 <= nb < 22:
                    # wo^T transposes interleaved mid-stage-1 (ps3 psum slots
                    # are idle here), off the stage-2 critical path
                    hb_, cb_ = divmod(nb - 14, CB)
                    tp = psB.tile([P, NTW], F32, tag="ps3", name="tp")
                    nc.tensor.transpose(
                        tp[:, :P], wo_sb[:, cb_, hb_ * P:(hb_ + 1) * P], ident
                    )
                    nc.vector.tensor_copy(
                        woT[:, hb_, cb_ * P:(cb_ + 1) * P], tp[:, :P]
                    )
                if nb == 20:
                    # rounded stage-2 operand copies land mid-stage-1 (DVE has
                    # slack here), off the stage-2 critical path
                    nc.vector.tensor_copy(wq_r, wq_sb)
                    nc.vector.tensor_copy(bq_r[:, :, 0], bq_sb)
                    nc.vector.tensor_scalar_mul(bq_r[:, :, 1], bq_sb, 0.0)
                vn = P if nb < NBLK - 1 else N - (NBLK - 1) * P  # 64 on last block
                ps = psA.tile([P, 512], F32, tag="mm512", name="ps")
                for cb in range(CB):
                    nc.tensor.matmul(
                        ps,
                        xfr[:, cb, nb * P:(nb + 1) * P],
                        wkvT[:, cb, :],
                        start=(cb == 0),
                        stop=(cb == CB - 1),
                    )
                if vn == P:
                    ktvt = wpool.tile([P, 2 * E], F32R, tag="ktvt")
                    nc.vector.tensor_add(ktvt, ps, bkv_full)
                else:
                    ktvt = ktvt_last
                    nc.vector.tensor_add(ktvt[:vn], ps[:vn], bkv_full[:vn])
                for eb in range(EB):
                    nc.tensor.matmul(
                        kv_ps[eb],
                        ktvt[:, eb * P:(eb + 1) * P],
                        ktvt[:, E:2 * E],
                        start=(nb == 0),
                        stop=(nb == NBLK - 1),
                    )



            # split psum evacuations across DVE and ACT, low (first-needed)
            # 128-col slices first, so the F matmuls start sooner
            kv_sb = cpool.tile([P, EB, E], F32R, tag="kv_sb")
            nc.vector.tensor_copy(kv_sb[:, 0, 0:P], kv_ps[0][:, 0:P])
            nc.scalar.activation(kv_sb[:, 1, 0:P], kv_ps[1][:, 0:P], ACT_COPY)
            nc.vector.tensor_copy(kv_sb[:, 0, P:E], kv_ps[0][:, P:E])
            nc.scalar.activation(kv_sb[:, 1, P:E], kv_ps[1][:, P:E], ACT_COPY)

            # ------------- stage 2: fold weights through KV -------------
            # F[eh, c] = sum_e KV[e, eh] wq[e, c], scaled by 1/N
            f_sb = cpool.tile([P, EB, C], F32R, tag="f_sb")
            for hb in range(EB):
                fps = psA.tile([P, 512], F32, tag="mm512", name="fps")
                for eb in range(EB):
                    nc.tensor.matmul(
                        fps,
                        kv_sb[:, eb, hb * P:(hb + 1) * P],
                        wq_r[:, eb, :],
                        start=(eb == 0),
                        stop=(eb == EB - 1),
                    )
                if hb == 0:
                    nc.vector.tensor_scalar_mul(f_sb[:, hb, 0:P], fps[:, 0:P], 1.0 / N)
                    nc.vector.tensor_scalar_mul(f_sb[:, hb, P:C], fps[:, P:C], 1.0 / N)
                else:
                    nc.scalar.activation(
                        f_sb[:, hb, 0:P], fps[:, 0:P], ACT_COPY, scale=1.0 / N
                    )
                    nc.scalar.activation(
                        f_sb[:, hb, P:C], fps[:, P:C], ACT_COPY, scale=1.0 / N
                    )


            # t[eh] = sum_e KV[e, eh] bq[e] (2-wide: fp32r needs even
            # moving element count); emitted before W1T so the tiny matmuls
            # fill PE waits on the f_sb evacuations
            t_sb = cpool.tile([P, EB, 2], F32R, tag="t_sb")
            for hb in range(EB):
                tps = psA.tile([P, 512], F32, tag="mm512", name="tps")
                for eb in range(EB):
                    nc.tensor.matmul(
                        tps[:, 0:2],
                        kv_sb[:, eb, hb * P:(hb + 1) * P],
                        bq_r[:, eb, :],
                        start=(eb == 0),
                        stop=(eb == EB - 1),
                    )
                nc.vector.tensor_copy(t_sb[:, hb, :], tps[:, 0:2])

            # W1T[c, c'] = sum_eh F[eh, c] woT[eh, c']
            w1t = cpool.tile([P, CB, C], F32R, tag="w1t")
            for cb in range(CB):
                wps = psA.tile([P, 512], F32, tag="mm512", name="wps")
                for hb in range(EB):
                    nc.tensor.matmul(
                        wps,
                        f_sb[:, hb, cb * P:(cb + 1) * P],
                        woT[:, hb, :],
                        start=(hb == 0),
                        stop=(hb == EB - 1),
                    )
                if cb % 2 == 0:
                    nc.vector.tensor_copy(w1t[:, cb, 0:P], wps[:, 0:P])
                    nc.vector.tensor_copy(w1t[:, cb, P:C], wps[:, P:C])
                else:
                    nc.scalar.activation(w1t[:, cb, 0:P], wps[:, 0:P], ACT_COPY)
                    nc.scalar.activation(w1t[:, cb, P:C], wps[:, P:C], ACT_COPY)
            bias_out = cpool.tile([P, CB], F32, tag="bias_out")
            for cb in range(CB):
                cps = psA.tile([P, 512], F32, tag="mm512", name="cps")
                for hb in range(EB):
                    nc.tensor.matmul(
                        cps[:, 0:2],
                        woT[:, hb, cb * P:(cb + 1) * P],
                        t_sb[:, hb, :],
                        start=(hb == 0),
                        stop=(hb == EB - 1),
                    )
                nc.scalar.activation(
                    bias_out[:, cb:cb + 1],
                    cps[:, 0:1],
                    ACT_IDENT,
                    bias=bo_sb[:, cb:cb + 1],
                    scale=1.0 / N,
                )

            # ------------- stage 3: out = W1 @ x + bias + x -------------
            for cpb in range(CB):
                for t in range(NT):
                    sl = slice(t * NTW, (t + 1) * NTW)
                    ps3 = psB.tile([P, NTW], F32, tag="ps3", name="ps3")
                    for cb in range(CB):
                        nc.tensor.matmul(
                            ps3,
                            w1t[:, cb, cpb * P:(cpb + 1) * P],
                            xfr[:, cb, sl],
                            start=(cb == 0),
                            stop=(cb == CB - 1),
                        )
                    ot = wpool.tile([P, NTW], F32, tag="ot", bufs=6)
                    nc.scalar.activation(
                        ot, ps3, ACT_IDENT, bias=bias_out[:, cpb:cpb + 1]
                    )
                    nc.vector.tensor_add(ot, ot, xf[:, cpb, sl])
                    nc.sync.dma_start(our[:, cpb, sl], ot)

    nc.compile()
    return nc


_NC_CACHE = None


def kernel(**inputs) -> np.ndarray:
    global _NC_CACHE
    x = np.ascontiguousarray(inputs["x"], dtype=np.float32)
    B = x.shape[0]
    weights = {
        k: np.ascontiguousarray(inputs[k], dtype=np.float32)
        for k in ("wq", "bq", "wk", "bk", "wv", "bv", "wo", "bo")
    }

    if _NC_CACHE is None:
        _NC_CACHE = build_kernel()
    nc = _NC_CACHE

    in_maps = []
    for b in range(B):
        m = {"x": x[b].reshape(C, N)}
        m.update(weights)
        in_maps.append(m)

    # The axon terminal occasionally reports a transient
    # NRT_EXEC_UNIT_UNRECOVERABLE on a session's first touch. In-process
    # retries rarely recover (the PJRT session stays poisoned), so after a
    # short retry fall back to re-running in a fresh subprocess.
    res = None
    for attempt in range(2):
        try:
            res = run_bass_kernel_spmd(nc, in_maps, core_ids=list(range(B)))
            break
        except Exception:
            if attempt == 1:
                return _kernel_subprocess(x, weights)
            import time

            time.sleep(10)
    out = np.stack([res.results[b]["out"] for b in range(B)], axis=0)
    return out.reshape(B, C, 56, 56)


def _kernel_subprocess(x, weights, depth=0):
    """Run the kernel in a fresh python process (fresh axon session)."""
    import os
    import subprocess
    import sys
    import tempfile

    if depth >= 2:
        raise RuntimeError("kernel failed repeatedly in subprocess retries")
    d = tempfile.mkdtemp()
    inp = os.path.join(d, "in.npz")
    outp = os.path.join(d, "out.npy")
    np.savez(inp, x=x, **weights)
    script = (
        "import numpy as np, sys; sys.path.insert(0, %r); "
        "import kernel; d = dict(np.load(%r)); "
        "np.save(%r, kernel.kernel(**d))"
        % (os.path.dirname(os.path.abspath(__file__)), inp, outp)
    )
    for attempt in range(3):
        r = subprocess.run([sys.executable, "-c", script], capture_output=True)
        if r.returncode == 0 and os.path.exists(outp):
            return np.load(outp)
        import time

        time.sleep(10)
    raise RuntimeError(
        "kernel subprocess failed: %s" % r.stderr.decode()[-2000:]
    )


if __name__ == "__main__":
    import sys

    if "--compile-only" in sys.argv:
        import tempfile

        from concourse.bass_utils import compile_bass_kernel

        nc = build_kernel()
        d = tempfile.mkdtemp()
        path = compile_bass_kernel(nc, d)
        print("compiled OK:", path)
    else:
        inputs = dict(np.load("/tmp/nl_inputs.npz"))
        got = kernel(**inputs)
        print("kernel output shape:", got.shape)


# revision 71
# speedup vs baseline: 1.0115x; 1.0115x over previous
"""DotProductNonLocalBlock Trainium2 kernel.

Reference computation (per batch b, with xf = x[b] reshaped (C, N)):
    Q = wq @ xf + bq ; K = wk @ xf + bk ; V = wv @ xf + bv      (E x N each)
    attn  = (Q^T K) / N                                          (N x N)
    embed = attn @ V^T                                           (N x E)
    out   = wo @ embed^T + bo + xf                               (C x N)

There is no softmax, so matmul associativity collapses the N x N
attention matrix:
    KV   = K @ V^T                          (E x E)
    out  = wo @ (KV^T @ Q) / N + bo + xf
         = W1 @ xf + cvec + xf,   W1 = wo @ KV^T @ wq / N  (C x C)
         with cvec = wo @ (KV^T @ bq) / N + bo
This turns ~13.4 GFLOP/batch into ~3.7 GFLOP/batch of dense matmuls.

Matmuls run in float32r (full PE rate; plain fp32 is 4 cycles/row).
float32r operands must be produced by a rounding instruction, so every
matmul input tile is written by DVE/ACT with a float32r output dtype.

Sharding: data-parallel over batch, one batch per NeuronCore (8 cores).
"""

import numpy as np

import concourse.bass as bass
from concourse import bacc
import concourse.mybir as mybir
import concourse.tile as tile
from concourse.bass_utils import run_bass_kernel_spmd
from concourse.masks import make_identity

F32 = mybir.dt.float32
F32R = mybir.dt.float32r
ACT_COPY = mybir.ActivationFunctionType.Copy
ACT_IDENT = mybir.ActivationFunctionType.Identity

P = 128          # partitions
C = 512          # in_channels
E = 256          # embed dim
N = 3136         # 56*56 pixels
NP = 3200        # N padded to 25*128
CB = C // P      # 4 channel blocks
EB = E // P      # 2 embed blocks
NBLK = NP // P   # 25 pixel blocks of 128
NT = 7           # pixel tiles of 448 for projection-style matmuls
NTW = N // NT    # 448


def build_kernel():
    nc = bacc.Bacc(None, target_bir_lowering=False)

    x_d = nc.dram_tensor("x", (C, N), F32, kind="ExternalInput")
    wq_d = nc.dram_tensor("wq", (E, C), F32, kind="ExternalInput")
    bq_d = nc.dram_tensor("bq", (E,), F32, kind="ExternalInput")
    wk_d = nc.dram_tensor("wk", (E, C), F32, kind="ExternalInput")
    bk_d = nc.dram_tensor("bk", (E,), F32, kind="ExternalInput")
    wv_d = nc.dram_tensor("wv", (E, C), F32, kind="ExternalInput")
    bv_d = nc.dram_tensor("bv", (E,), F32, kind="ExternalInput")
    wo_d = nc.dram_tensor("wo", (C, E), F32, kind="ExternalInput")
    bo_d = nc.dram_tensor("bo", (C,), F32, kind="ExternalInput")
    out_d = nc.dram_tensor("out", (C, N), F32, kind="ExternalOutput")

    xr = x_d[:, :].rearrange("(cb p) n -> p cb n", p=P)
    our = out_d[:, :].rearrange("(cb p) n -> p cb n", p=P)

    with tile.TileContext(nc) as tc:
        with (
            tc.tile_pool(name="const", bufs=1) as cpool,
            tc.tile_pool(name="work", bufs=3) as wpool,
            tc.tile_pool(name="psA", bufs=3, space="PSUM") as psA,
            tc.tile_pool(name="psB", bufs=3, space="PSUM") as psB,
        ):
            # ---------------- constants & inputs in SBUF ----------------
            # bkv_row memset must precede ident/e0 so the Pool-observing
            # transpose below covers its tick too (PE matmuls allow 1 wait).
            bkv_row = cpool.tile([P, 2 * E], F32, tag="bkv_row")
            nc.gpsimd.memset(bkv_row, 0.0)
            ident = cpool.tile([P, P], F32, tag="ident")
            make_identity(nc, ident)

            # DMA issue order matters: the shared DMA backend serializes, so
            # load what gates compute first (wk/wv for the weight transposes,
            # then x in small leading chunks), and stage-2-only weights last.
            # x tiles declared up front; loads are interleaved with the
            # weight DMAs below (the shared DMA backend serializes, so the
            # issue order IS the arrival order).
            xf = cpool.tile([P, CB, NP], F32, tag="xf")
            xfr = cpool.tile([P, CB, NP], F32R, tag="xfr")
            nc.gpsimd.memset(xf[:, :, N:NP], 0.0)
            nc.scalar.activation(xfr[:, :, N:NP], xf[:, :, N:NP], ACT_COPY)

            def load_x(a, b):
                nc.sync.dma_start(xf[:, :, a:b], xr[:, :, a:b])
                # rounding copy on ACT: keeps DVE free for stage-1 evacuations
                nc.scalar.activation(xfr[:, :, a:b], xf[:, :, a:b], ACT_COPY)

            wk_nat = cpool.tile([P, EB, C], F32, tag="wk_nat")
            wv_nat = cpool.tile([P, EB, C], F32, tag="wv_nat")
            for eb in range(EB):
                nc.sync.dma_start(
                    wk_nat[:, eb], wk_d[:, :].rearrange("(eb p) c -> p eb c", p=P)[:, eb]
                )
            load_x(0, 2 * P)
            for eb in range(EB):
                nc.sync.dma_start(
                    wv_nat[:, eb], wv_d[:, :].rearrange("(eb p) c -> p eb c", p=P)[:, eb]
                )
            load_x(2 * P, 4 * P)
            nc.sync.dma_start(bkv_row[0:1, 0:E], bk_d[:].rearrange("(o e) -> o e", o=1))
            nc.sync.dma_start(bkv_row[0:1, E:2 * E], bv_d[:].rearrange("(o e) -> o e", o=1))
            # first two 448 chunks in halves: their leading data arrives
            # ~1.3us sooner, feeding stage-1 blocks 4-7 without a stall
            load_x(512, 736)
            load_x(736, 960)
            load_x(960, 1184)
            load_x(1184, 1408)
            for i in range(2, 4):
                a = 512 + i * NTW
                b = min(a + NTW, N)
                load_x(a, b)
            # wo ahead of the last x chunks: it feeds the woT transposes
            # interleaved into mid-stage-1
            wo_sb = cpool.tile([P, CB, E], F32, tag="wo_sb")
            nc.sync.dma_start(wo_sb, wo_d[:, :].rearrange("(cb p) e -> p cb e", p=P))
            for i in range(4, 6):
                a = 512 + i * NTW
                b = min(a + NTW, N)
                load_x(a, b)

            wq_sb = cpool.tile([P, EB, C], F32, tag="wq_sb")
            nc.sync.dma_start(wq_sb, wq_d[:, :].rearrange("(eb p) c -> p eb c", p=P))
            bq_sb = cpool.tile([P, EB], F32, tag="bq_sb")
            nc.sync.dma_start(bq_sb, bq_d[:].rearrange("(eb p) -> p eb", p=P))
            bo_sb = cpool.tile([P, CB], F32, tag="bo_sb")
            nc.sync.dma_start(bo_sb, bo_d[:].rearrange("(cb p) -> p cb", p=P))

            # ------------- weight transposes via PE (fp32) -------------
            # wkvT[c, cb, 0:E]=wk^T, [c, cb, E:2E]=wv^T  (contraction layout)
            wkvT = cpool.tile([P, CB, 2 * E], F32R, tag="wkvT")

            def transpose_into(src, off):
                for eb in range(EB):
                    for cb in range(CB):
                        # borrow the (idle) stage-3 psum slots: 4-deep rotation
                        # keeps the transpose->copy pipeline streaming
                        tp = psB.tile([P, NTW], F32, tag="ps3", name="tp")
                        nc.tensor.transpose(
                            tp[:, :P], src[:, eb, cb * P:(cb + 1) * P], ident
                        )
                        dst = wkvT[:, cb, off + eb * P:off + (eb + 1) * P]
                        if cb % 2 == 0:
                            nc.vector.tensor_copy(dst, tp[:, :P])
                        else:
                            nc.scalar.activation(dst, tp[:, :P], ACT_COPY)

            transpose_into(wk_nat, 0)
            transpose_into(wv_nat, E)

            # [bk|bv] replicated on all partitions via GpSimd broadcast
            # (keeps PE free and off the bias-DMA dependency)
            bkv_full = cpool.tile([P, 2 * E], F32, tag="bkv_full")
            nc.gpsimd.partition_broadcast(bkv_full, bkv_row[0:1, :])

            # ------------- stage 1: K^T/V^T blocks + KV accumulation -------------
            # For each 128-pixel block: psum[n, 0:E]=K^T, [n, E:2E]=V^T
            # then KV[e, eh] += KT_blk^T @ VT_blk  accumulated in PSUM.
            kv_ps = []
            for eb in range(EB):
                kvt = psA.tile([P, E], F32, tag="kv_ps", bufs=EB, name="kvt")
                kv_ps.append(kvt)
            woT = cpool.tile([P, EB, C], F32R, tag="woT")
            wq_r = cpool.tile([P, EB, C], F32R, tag="wq_r")
            bq_r = cpool.tile([P, EB, 2], F32R, tag="bq_r")
            # the last pixel block has only 64 valid rows; its padded rows
            # must be exactly zero for KV. Pre-zero a dedicated tile early so
            # only the valid-row add sits on the end-of-stage-1 critical path.
            ktvt_last = cpool.tile([P, 2 * E], F32R, tag="ktvt_last")
            nc.vector.tensor_scalar_mul(ktvt_last[64:], bkv_full[64:], 0.0)
            for nb in range(NBLK):
                if 14 <= nb < 22:
                    # wo^T transposes interleaved mid-stage-1 (ps3 psum slots
                    # are idle here), off the stage-2 critical path
                    hb_, cb_ = divmod(nb - 14, CB)
                    tp = psB.tile([P, NTW], F32, tag="ps3", name="tp")
                    nc.tensor.transpose(
                        tp[:, :P], wo_sb[:, cb_, hb_ * P:(hb_ + 1) * P], ident
                    )
                    nc.vector.tensor_copy(
                        woT[:, hb_, cb_ * P:(cb_ + 1) * P], tp[:, :P]
                    )
                if nb == 20:
                    # rounded stage-2 operand copies land mid-stage-1 (DVE has
                    # slack here), off the stage-2 critical path
                    nc.vector.tensor_copy(wq_r, wq_sb)
                    nc.vector.tensor_copy(bq_r[:, :, 0], bq_sb)
                    nc.vector.tensor_scalar_mul(bq_r[:, :, 1], bq_sb, 0.0)
                vn = P if nb < NBLK - 1 else N - (NBLK - 1) * P  # 64 on last block
                ps = psA.tile([P, 512], F32, tag="mm512", name="ps")
                for cb in range(CB):
                    nc.tensor.matmul(
                        ps,
                        xfr[:, cb, nb * P:(nb + 1) * P],
                        wkvT[:, cb, :],
                        start=(cb == 0),
                        stop=(cb == CB - 1),
                    )
                if vn == P:
                    ktvt = wpool.tile([P, 2 * E], F32R, tag="ktvt")
                    nc.vector.tensor_add(ktvt, ps, bkv_full)
                else:
                    ktvt = ktvt_last
                    nc.vector.tensor_add(ktvt[:vn], ps[:vn], bkv_full[:vn])
                for eb in range(EB):
                    nc.tensor.matmul(
                        kv_ps[eb],
                        ktvt[:, eb * P:(eb + 1) * P],
                        ktvt[:, E:2 * E],
                        start=(nb == 0),
                        stop=(nb == NBLK - 1),
                    )



            # split psum evacuations across DVE and ACT, low (first-needed)
            # 128-col slices first, so the F matmuls start sooner
            kv_sb = cpool.tile([P, EB, E], F32R, tag="kv_sb")
            nc.vector.tensor_copy(kv_sb[:, 0, 0:P], kv_ps[0][:, 0:P])
            nc.scalar.activation(kv_sb[:, 1, 0:P], kv_ps[1][:, 0:P], ACT_COPY)
            nc.vector.tensor_copy(kv_sb[:, 0, P:E], kv_ps[0][:, P:E])
            nc.scalar.activation(kv_sb[:, 1, P:E], kv_ps[1][:, P:E], ACT_COPY)

            # ------------- stage 2: fold weights through KV -------------
            # F[eh, c] = sum_e KV[e, eh] wq[e, c], scaled by 1/N
            f_sb = cpool.tile([P, EB, C], F32R, tag="f_sb")
            for hb in range(EB):
                fps = psA.tile([P, 512], F32, tag="mm512", name="fps")
                for eb in range(EB):
                    nc.tensor.matmul(
                        fps,
                        kv_sb[:, eb, hb * P:(hb + 1) * P],
                        wq_r[:, eb, :],
                        start=(eb == 0),
                        stop=(eb == EB - 1),
                    )
                if hb == 0:
                    nc.vector.tensor_scalar_mul(f_sb[:, hb, 0:P], fps[:, 0:P], 1.0 / N)
                    nc.vector.tensor_scalar_mul(f_sb[:, hb, P:C], fps[:, P:C], 1.0 / N)
                else:
                    nc.scalar.activation(
                        f_sb[:, hb, 0:P], fps[:, 0:P], ACT_COPY, scale=1.0 / N
                    )
                    nc.scalar.activation(
                        f_sb[:, hb, P:C], fps[:, P:C], ACT_COPY, scale=1.0 / N
                    )


            # t[eh] = sum_e KV[e, eh] bq[e] (2-wide: fp32r needs even
            # moving element count); emitted before W1T so the tiny matmuls
            # fill PE waits on the f_sb evacuations
            t_sb = cpool.tile([P, EB, 2], F32R, tag="t_sb")
            for hb in range(EB):
                tps = psA.tile([P, 512], F32, tag="mm512", name="tps")
                for eb in range(EB):
                    nc.tensor.matmul(
                        tps[:, 0:2],
                        kv_sb[:, eb, hb * P:(hb + 1) * P],
                        bq_r[:, eb, :],
                        start=(eb == 0),
                        stop=(eb == EB - 1),
                    )
                nc.vector.tensor_copy(t_sb[:, hb, :], tps[:, 0:2])

            # W1T[c, c'] = sum_eh F[eh, c] woT[eh, c']
            w1t = cpool.tile([P, CB, C], F32R, tag="w1t")
            for cb in range(CB):
                wps = psA.tile([P, 512], F32, tag="mm512", name="wps")
                for hb in range(EB):
                    nc.tensor.matmul(
                        wps,
                        f_sb[:, hb, cb * P:(cb + 1) * P],
                        woT[:, hb, :],
                        start=(hb == 0),
                        stop=(hb == EB - 1),
                    )
                if cb % 2 == 0:
                    nc.vector.tensor_copy(w1t[:, cb, 0:P], wps[:, 0:P])
                    nc.vector.tensor_copy(w1t[:, cb, P:C], wps[:, P:C])
                else:
                    nc.scalar.activation(w1t[:, cb, 0:P], wps[:, 0:P], ACT_COPY)
                    nc.scalar.activation(w1t[:, cb, P:C], wps[:, P:C], ACT_COPY)
            bias_out = cpool.tile([P, CB], F32, tag="bias_out")
            for cb in range(CB):
                cps = psA.tile([P, 512], F32, tag="mm512", name="cps")
                for hb in range(EB):
                    nc.tensor.matmul(
                        cps[:, 0:2],
                        woT[:, hb, cb * P:(cb + 1) * P],
                        t_sb[:, hb, :],
                        start=(hb == 0),
                        stop=(hb == EB - 1),
                    )
                nc.scalar.activation(
                    bias_out[:, cb:cb + 1],
                    cps[:, 0:1],
                    ACT_IDENT,
                    bias=bo_sb[:, cb:cb + 1],
                    scale=1.0 / N,
                )

            # ------------- stage 3: out = W1 @ x + bias + x -------------
            for cpb in range(CB):
                for t in range(NT):
                    sl = slice(t * NTW, (t + 1) * NTW)
                    ps3 = psB.tile([P, NTW], F32, tag="ps3", name="ps3")
                    for cb in range(CB):
                        nc.tensor.matmul(
                            ps3,
                            w1t[:, cb, cpb * P:(cpb + 1) * P],
                            xfr[:, cb, sl],
                            start=(cb == 0),
                            stop=(cb == CB - 1),
                        )
                    ot = wpool.tile([P, NTW], F32, tag="ot", bufs=6)
                    nc.scalar.activation(
                        ot, ps3, ACT_IDENT, bias=bias_out[:, cpb:cpb + 1]
                    )
                    nc.vector.tensor_add(ot, ot, xf[:, cpb, sl])
                    nc.sync.dma_start(our[:, cpb, sl], ot)

    nc.compile()
    return nc


_NC_CACHE = None


def kernel(**inputs) -> np.ndarray:
    global _NC_CACHE
    x = np.ascontiguousarray(inputs["x"], dtype=np.float32)
    B = x.shape[0]
    weights = {
        k: np.ascontiguousarray(inputs[k], dtype=np.float32)
        for k in ("wq", "bq", "wk", "bk", "wv", "bv", "wo", "bo")
    }

    if _NC_CACHE is None:
        _NC_CACHE = build_kernel()
    nc = _NC_CACHE

    in_maps = []
    for b in range(B):
        m = {"x": x[b].reshape(C, N)}
        m.update(weights)
        in_maps.append(m)

    # The axon terminal occasionally reports a transient
    # NRT_EXEC_UNIT_UNRECOVERABLE on a session's first touch. In-process
    # retries rarely recover (the PJRT session stays poisoned), so after a
    # short retry fall back to re-running in a fresh subprocess.
    res = None
    for attempt in range(2):
        try:
            res = run_bass_kernel_spmd(nc, in_maps, core_ids=list(range(B)))
            break
        except Exception:
            if attempt == 1:
                return _kernel_subprocess(x, weights)
            import time

            time.sleep(10)
    out = np.stack([res.results[b]["out"] for b in range(B)], axis=0)
    return out.reshape(B, C, 56, 56)


def _kernel_subprocess(x, weights, depth=0):
    """Run the kernel in a fresh python process (fresh axon session)."""
    import os
    import subprocess
    import sys
    import tempfile

    if depth >= 2:
        raise RuntimeError("kernel failed repeatedly in subprocess retries")
    d = tempfile.mkdtemp()
    inp = os.path.join(d, "in.npz")
    outp = os.path.join(d, "out.npy")
    np.savez(inp, x=x, **weights)
    script = (
        "import numpy as np, sys; sys.path.insert(0, %r); "
        "import kernel; d = dict(np.load(%r)); "
        "np.save(%r, kernel.kernel(**d))"
        % (os.path.dirname(os.path.abspath(__file__)), inp, outp)
    )
    for attempt in range(3):
        r = subprocess.run([sys.executable, "-c", script], capture_output=True)
        if r.returncode == 0 and os.path.exists(outp):
            return np.load(outp)
        import time

        time.sleep(10)
    raise RuntimeError(
        "kernel subprocess failed: %s" % r.stderr.decode()[-2000:]
    )


if __name__ == "__main__":
    import sys

    if "--compile-only" in sys.argv:
        import tempfile

        from concourse.bass_utils import compile_bass_kernel

        nc = build_kernel()
        d = tempfile.mkdtemp()
        path = compile_bass_kernel(nc, d)
        print("compiled OK:", path)
    else:
        inputs = dict(np.load("/tmp/nl_inputs.npz"))
        got = kernel(**inputs)
        print("kernel output shape:", got.shape)
